# revision 16
# baseline (speedup 1.0000x reference)
"""Trainium2 Bass kernel for nn_ConditionalFeedForward (MoE top-2 FFN).

Strategy: pair tensor-parallel over the intermediate dim, expert-parallel
over pairs (TP2 x EP4). Experts are greedily paired by token load; pair j
lives on cores (2j, 2j+1): core 2j holds the LOW half of both experts'
intermediate dim (gate rows [0:I/2], up rows [0:I/2], w2 cols [0:I/2]),
core 2j+1 the HIGH half. Both cores of a pair process the same token sets;
each returns a PARTIAL output (its half of the I-contraction in GEMM2) and
the host adds the two partials. Routing/gather/scatter stays on the host.

Why TP2: with plain EP (expert e on core e) every core pays the MAXIMUM
expert token count C_max in matmul columns. With TP2 the two free-dim
classes are (max of the 4 small experts, max of the 4 big experts) --
e.g. 252+274=526 columns vs 2x274=548 -- a ~4% PE-cycle cut, while the
per-core weight traffic is unchanged (half of two experts = one expert).

Per core, with (C1, C2) = capacities of its two experts (e1, e2):
    h13T_e = w13[e][half] @ xgT_e     # [I, Cx] rows of gate|up half
    hT_e   = silu(gate) * up         # [I/2, Cx]
    partT_e = w2[e][:, half] @ hT_e  # [D, Cx] partial (half contraction)
Everything is feature-major ([feature, token]) so both GEMMs use the weight
as the stationary operand and never need an on-device transpose.

Weights and activations are bf16 (fp32 PSUM accumulation). Measured
rooflines (per core): PE 384 matmuls x (C1+C2)/2 cols at the data-dependent
clock (~2.0-2.06 GHz under real bf16 toggle load due to the power/P0
governor; 2.4 GHz only on low-toggle data); HBM stream ~14.3 MB at
~352 GB/s. The kernel is PE-bound at the power-limited clock; fp8
(DoubleRow) would halve cycles but fails the 2e-2 accuracy gate (measured
8.5e-2 in simulation), so bf16 is the floor.

Input DMAs live on Sync/Scalar (HWDGE, alternating); under
For_i(staggered_reset) they prefetch iteration i+1 while iteration i's
GEMM2 is on the PE. UNROLL=4 with PE-only branch hints; wpool bufs=6.
"""

from contextlib import ExitStack

import ml_dtypes
import numpy as np

import concourse.bass as bass
import concourse.mybir as mybir
import concourse.tile as tile
from concourse import bacc
from concourse.bass_utils import run_bass_kernel_spmd

# Problem shape (hardcoded per harness contract).
E = 8          # experts
D = 1024       # model dim
I = 2048       # intermediate dim
P = 128        # SBUF partitions
KD = D // P    # 8 k-tiles over D
MPH = I // P // 2       # 8 gate|up pair-panels per I-half
NW13 = MPH // 2         # 4 w13 DMA tiles per expert-half (2 panels each)
MO = D // P             # 8 output row tiles
KIH = I // P // 2       # 8 k-tiles over the I-half
NW2 = MO // 2           # 4 w2 DMA tiles per expert-half (2 out-panels each)
PAIRC = KD * 2 * P      # columns of one gate|up fused panel
W13T = 2 * PAIRC        # columns of one w13 DMA tile (2 panels)
W2T = 2 * KIH * P       # columns of one w2 DMA tile (2 out-panels)

F32 = mybir.dt.float32
BF16 = mybir.dt.bfloat16
NP_BF16 = ml_dtypes.bfloat16


def build_program(C1: int, C2: int, repeats: int = 1, hw_loop: bool = False):
    """Build + compile the SPMD per-core program for capacities (C1, C2)."""
    nc = bacc.Bacc(
        "TRN2", target_bir_lowering=False, debug=False, num_devices=E
    )
    CS = C1 + C2
    xg_d = nc.dram_tensor("xg", [P, KD * CS], BF16, kind="ExternalInput").ap()
    w13p_d = nc.dram_tensor(
        "w13p", [P, 2 * NW13 * W13T], BF16, kind="ExternalInput"
    ).ap()
    w2p_d = nc.dram_tensor(
        "w2p", [P, 2 * NW2 * W2T], BF16, kind="ExternalInput"
    ).ap()
    out_d = nc.dram_tensor(
        "outt", [P, MO * CS], BF16, kind="ExternalOutput"
    ).ap()

    with tile.TileContext(nc) as tc, ExitStack() as ctx:
        resident = ctx.enter_context(tc.tile_pool(name="resident", bufs=1))
        psum = ctx.enter_context(tc.tile_pool(name="psum", bufs=4, space="PSUM"))
        spool = ctx.enter_context(tc.tile_pool(name="s", bufs=2))
        hpool = ctx.enter_context(tc.tile_pool(name="h", bufs=2))
        wpool = ctx.enter_context(tc.tile_pool(name="w", bufs=6))

        _emit_warmup(nc, resident, psum, spool, max(C1, C2))
        if hw_loop and repeats > 1:
            UNROLL = 4
            hint_engs = (mybir.EngineType.PE,)
            n_loop = (repeats - 1) // UNROLL
            leftover = repeats - n_loop * UNROLL
            if n_loop > 0:
                label = "moe_backedge"
                with tc.For_i(0, n_loop, 1,
                              hint_engines=hint_engs,
                              back_edge_label=label,
                              staggered_reset=True):
                    for u in range(UNROLL):
                        if u == UNROLL - 1:
                            tc.mark_branch_hint_location(
                                label, engines=hint_engs
                            )
                        _emit_body(nc, tc, hpool, wpool, psum, spool,
                                   xg_d, w13p_d, w2p_d, out_d, C1, C2)
            for _ in range(leftover):
                _emit_body(nc, tc, hpool, wpool, psum, spool,
                           xg_d, w13p_d, w2p_d, out_d, C1, C2)
        else:
            for _ in range(repeats):
                _emit_body(nc, tc, hpool, wpool, psum, spool,
                           xg_d, w13p_d, w2p_d, out_d, C1, C2)

    nc.compile()
    return nc


def _emit_warmup(nc, resident, psum, spool, C):
    # PE warm-up: dummy matmuls on a zeroed tile run while the first DMAs
    # land, pushing the PE clock-gate (HAM) to full rate before the real
    # matmul stream starts. A dummy Silu loads the ACT function table.
    warm = resident.tile([P, 512], BF16, tag="warm")
    nc.gpsimd.memset(warm[:], 0.0)
    pwarm = psum.tile([P, C], F32, tag="pg", bufs=2)
    for _ in range(8):
        nc.tensor.matmul(
            pwarm[:], lhsT=warm[:, :P], rhs=warm[:, :C], start=True, stop=True
        )
    sil_warm = spool.tile([P, 1], F32, tag="sil_warm")
    nc.scalar.activation(
        sil_warm[:], warm[:, :1], mybir.ActivationFunctionType.Silu
    )


def _emit_body(nc, tc, hpool, wpool, psum, spool,
               xg_d, w13p_d, w2p_d, out_d, C1, C2):
    CS = C1 + C2
    caps = (C1, C2)
    xg_off = (0, KD * C1)           # column offset of each expert's xg block
    hT_off = (0, KIH * C1)          # offset of each expert's hT block
    ot_off = (0, MO * C1)           # offset of each expert's output block

    hT_all = hpool.tile([P, KIH * CS], BF16, tag="hT_all")
    xgt = hpool.tile([P, KD * CS], BF16, tag="xgt")
    nc.sync.dma_start(out=xgt[:, : KD * C1], in_=xg_d[:, : KD * C1])
    nc.sync.dma_start(out=xgt[:, KD * C1 :], in_=xg_d[:, KD * C1 :])

    # GEMM1 + SiLU*up. 8 w13 DMA tiles: t in 0..3 -> expert 1 (free dim C1),
    # t in 4..7 -> expert 2 (free dim C2). Each tile = 2 fused gate|up panels.
    w2tiles = {}
    for t in range(8):
        ei = t // 4
        Cx = caps[ei]
        xb = xg_off[ei]
        wt = wpool.tile([P, W13T], BF16, tag="w13")
        if t == 0:
            # Split tile 0 across both HWDGE rings so its first half (and
            # the first matmuls) are ready sooner.
            nc.sync.dma_start(out=wt[:, :PAIRC], in_=w13p_d[:, :PAIRC])
            nc.scalar.dma_start(out=wt[:, PAIRC:], in_=w13p_d[:, PAIRC:W13T])
        else:
            eng = nc.sync if t % 2 == 0 else nc.scalar
            eng.dma_start(out=wt[:], in_=w13p_d[:, t * W13T : (t + 1) * W13T])
        for h in range(2):
            p = 2 * (t % 4) + h     # panel index within this expert's half
            base = h * PAIRC
            psg = psum.tile([P, Cx], F32, tag="pg", bufs=2)
            psu = psum.tile([P, Cx], F32, tag="pt", bufs=2)
            for k in range(KD):
                nc.tensor.matmul(
                    psg[:],
                    lhsT=wt[:, base + k * 2 * P : base + k * 2 * P + P],
                    rhs=xgt[:, xb + k * Cx : xb + (k + 1) * Cx],
                    start=(k == 0),
                    stop=(k == KD - 1),
                )
            for k in range(KD):
                nc.tensor.matmul(
                    psu[:],
                    lhsT=wt[:, base + k * 2 * P + P : base + (k + 1) * 2 * P],
                    rhs=xgt[:, xb + k * Cx : xb + (k + 1) * Cx],
                    start=(k == 0),
                    stop=(k == KD - 1),
                )
            sil = spool.tile([P, Cx], F32, tag="sil", bufs=3)
            nc.scalar.activation(
                sil[:], psg[:], mybir.ActivationFunctionType.Silu
            )
            nc.vector.tensor_mul(
                hT_all[:, hT_off[ei] + p * Cx : hT_off[ei] + (p + 1) * Cx],
                sil[:], psu[:],
            )

        # Prefetch the 8 w2 tiles (2 per t) during the back half of GEMM1.
        if t >= 4:
            for s in range(2):
                jo = 2 * (t - 4) + s
                w2t = wpool.tile([P, W2T], BF16, tag="w2", bufs=12)
                eng = nc.sync if jo % 2 == 0 else nc.scalar
                eng.dma_start(
                    out=w2t[:], in_=w2p_d[:, jo * W2T : (jo + 1) * W2T]
                )
                w2tiles[jo] = w2t

    # GEMM2 (half contraction): w2 tiles jo 0..3 -> expert 1, 4..7 -> expert 2.
    # Each tile = 2 out-panels x KIH k-slices. Partial outT goes to ot8.
    ot8 = spool.tile([P, MO * CS], BF16, tag="ot8", bufs=2)
    for ei in range(2):
        Cx = caps[ei]
        hb = hT_off[ei]
        ob = ot_off[ei]
        for mo in range(MO):
            jo = 4 * ei + mo // 2
            h = mo % 2
            w2t = w2tiles[jo]
            base = h * KIH * P
            ps2 = psum.tile([P, Cx], F32, tag="po", bufs=4)
            for ki in range(KIH):
                nc.tensor.matmul(
                    ps2[:],
                    lhsT=w2t[:, base + ki * P : base + (ki + 1) * P],
                    rhs=hT_all[:, hb + ki * Cx : hb + (ki + 1) * Cx],
                    start=(ki == 0),
                    stop=(ki == KIH - 1),
                )
            nc.vector.tensor_copy(
                ot8[:, ob + mo * Cx : ob + (mo + 1) * Cx], ps2[:]
            )
        # Ship each expert's partial as soon as its evacuations finish:
        # expert 1's DMA drains during expert 2's GEMM2, so the scalar ring
        # is nearly empty at the iteration boundary (where next iteration's
        # w13 tile-0 half is queued behind it).
        nc.scalar.dma_start(
            out=out_d[:, ob : ob + MO * Cx],
            in_=ot8[:, ob : ob + MO * Cx],
        )


def _pack_xg(x, tok, C):
    """Feature-major k-tiled activation image [P, KD*C] for a token set."""
    xg = np.zeros((D, C), dtype=NP_BF16)
    if len(tok):
        xg[:, : len(tok)] = x[tok].T.astype(NP_BF16)
    return np.ascontiguousarray(
        xg.reshape(KD, P, C).transpose(1, 0, 2).reshape(P, KD * C)
    )


def _w13_half_tiles(w13e, h):
    """4 DMA tiles [P, W13T] for half h of one expert's w13 (fused gate|up)."""
    w13t = w13e.T.astype(NP_BF16)                    # [D, 2I]
    a = w13t.reshape(KD, P, 2 * (I // P), P)
    mp = I // P                                       # 16 gate (and up) panels
    fused = np.concatenate([a[:, :, :mp, :], a[:, :, mp:, :]], axis=-1)
    fused = fused.transpose(2, 1, 0, 3).reshape(mp, P, KD * 2 * P)
    half = fused[h * MPH : (h + 1) * MPH]             # 8 pair-panels
    return np.ascontiguousarray(
        half.reshape(NW13, 2, P, KD * 2 * P)
        .transpose(0, 2, 1, 3)
        .reshape(NW13, P, W13T)
    )


def _w2_half_tiles(w2e, h):
    """4 DMA tiles [P, W2T] for half h of one expert's w2."""
    w2t = w2e.T.astype(NP_BF16)                      # [I, D]
    half = w2t[h * (I // 2) : (h + 1) * (I // 2)]    # [I/2, D]
    b = half.reshape(KIH, P, MO, P)
    pan = b.transpose(2, 1, 0, 3).reshape(MO, P, KIH * P)
    return np.ascontiguousarray(
        pan.reshape(NW2, 2, P, KIH * P)
        .transpose(0, 2, 1, 3)
        .reshape(NW2, P, W2T)
    )


def prepare_core_inputs(x, expert_indices, w13, w2):
    """Host-side routing + TP2 packing.

    Returns (in_maps, pairs, slot_lists, C1, C2). Pair j = (e1, e2) lives on
    cores (2j, 2j+1); core 2j holds I-half 0, core 2j+1 I-half 1. Expert e1
    uses free dim C1, e2 uses C2.
    """
    x = np.asarray(x)
    flat_e = np.asarray(expert_indices).reshape(-1).astype(np.int64)
    slot_lists = [np.nonzero(flat_e == e)[0] for e in range(E)]
    counts = np.array([len(s) for s in slot_lists])
    order = np.argsort(-counts, kind="stable")
    # Greedy big-with-small pairing: (1st,8th), (2nd,7th), ... The small
    # expert goes first (class C1), the big one second (class C2).
    pairs = [(int(order[E - 1 - j]), int(order[j])) for j in range(E // 2)]
    align = lambda n: max(2, ((n + 1) // 2) * 2)
    C1 = align(max(counts[e1] for e1, _ in pairs))
    C2 = align(max(counts[e2] for _, e2 in pairs))

    w13 = np.asarray(w13)
    w2 = np.asarray(w2)
    in_maps = []
    for j in range(E // 2):
        e1, e2 = pairs[j]
        tok1 = slot_lists[e1] // 2
        tok2 = slot_lists[e2] // 2
        xg = np.concatenate(
            [_pack_xg(x, tok1, C1), _pack_xg(x, tok2, C2)], axis=1
        )
        for h in range(2):
            w13p = np.concatenate(
                [_w13_half_tiles(w13[e1], h), _w13_half_tiles(w13[e2], h)],
                axis=0,
            )  # [8, P, W13T]
            w13p = np.ascontiguousarray(
                w13p.transpose(1, 0, 2).reshape(P, 8 * W13T)
            )
            w2p = np.concatenate(
                [_w2_half_tiles(w2[e1], h), _w2_half_tiles(w2[e2], h)],
                axis=0,
            )  # [8, P, W2T]
            w2p = np.ascontiguousarray(
                w2p.transpose(1, 0, 2).reshape(P, 8 * W2T)
            )
            in_maps.append({"xg": xg, "w13p": w13p, "w2p": w2p})
    return in_maps, pairs, slot_lists, C1, C2


def assemble_output(results, pairs, slot_lists, C1, C2, T, dtype):
    out = np.zeros((T, D), dtype=dtype)
    caps = (C1, C2)
    for j in range(len(pairs)):
        lo = np.asarray(results[2 * j]["outt"], dtype=np.float32)
        hi = np.asarray(results[2 * j + 1]["outt"], dtype=np.float32)
        full = lo + hi                                  # [P, MO*(C1+C2)]
        off = 0
        for ei, e in enumerate(pairs[j]):
            Cx = caps[ei]
            slots = slot_lists[e]
            blk = full[:, off : off + MO * Cx]
            outt = blk.reshape(P, MO, Cx).transpose(1, 0, 2).reshape(D, Cx)
            if len(slots):
                out[slots] = outt[:, : len(slots)].T.astype(dtype)
            off += MO * Cx
    return out


def kernel(x, expert_indices, w13, w2):
    x = np.asarray(x)
    idx = np.asarray(expert_indices)
    T = idx.size
    flat_e = idx.reshape(-1).astype(np.int64)
    max_n = max(1, max((flat_e == e).sum() for e in range(E)))
    if max_n > 512:
        # Pathological imbalance: PSUM limits one pass to 512 tokens/expert.
        # Run the fixed-capacity program once per <=512-sized chunk round.
        out = np.zeros((T, D), dtype=x.dtype)
        slot_lists = [np.nonzero(flat_e == e)[0] for e in range(E)]
        chunked = [
            [s[i : i + 512] for i in range(0, max(len(s), 1), 512)]
            for s in slot_lists
        ]
        rounds = max(len(c) for c in chunked)
        for r in range(rounds):
            flat = np.full(T, -1, dtype=np.int64)
            for e, c in enumerate(chunked):
                if r < len(c):
                    flat[c[r]] = e
            sub_idx = flat.reshape(idx.shape)
            in_maps, pairs, sub_lists, C1, C2 = prepare_core_inputs(
                x, sub_idx, w13, w2
            )
            nc = build_program(C1, C2)
            res = _run_with_retry(nc, in_maps)
            part = assemble_output(
                res.results, pairs, sub_lists, C1, C2, T, x.dtype
            )
            mask = flat >= 0
            out[mask] = part[mask]
        return out
    in_maps, pairs, slot_lists, C1, C2 = prepare_core_inputs(
        x, idx, w13, w2
    )
    nc = build_program(C1, C2)
    res = _run_with_retry(nc, in_maps)
    return assemble_output(
        res.results, pairs, slot_lists, C1, C2, T, x.dtype
    )


def _run_with_retry(nc, in_maps, attempts=3):
    last_err = None
    for _ in range(attempts):
        try:
            return run_bass_kernel_spmd(nc, in_maps, core_ids=list(range(E)))
        except Exception as exc:  # intermittent NRT exec-unit wedge: retry
            last_err = exc
    raise last_err


# revision 17
# speedup vs baseline: 1.0207x; 1.0207x over previous
"""Trainium2 Bass kernel for nn_ConditionalFeedForward (MoE top-2 FFN).

Strategy: pair tensor-parallel over the intermediate dim, expert-parallel
over pairs (TP2 x EP4). Experts are greedily paired by token load; pair j
lives on cores (2j, 2j+1): core 2j holds the LOW half of both experts'
intermediate dim (gate rows [0:I/2], up rows [0:I/2], w2 cols [0:I/2]),
core 2j+1 the HIGH half. Both cores of a pair process the same token sets;
each returns a PARTIAL output (its half of the I-contraction in GEMM2) and
the host adds the two partials. Routing/gather/scatter stays on the host.

Why TP2: with plain EP (expert e on core e) every core pays the MAXIMUM
expert token count C_max in matmul columns. With TP2 the two free-dim
classes are (max of the 4 small experts, max of the 4 big experts) --
e.g. 252+274=526 columns vs 2x274=548 -- a ~4% PE-cycle cut, while the
per-core weight traffic is unchanged (half of two experts = one expert).

Per core, with (C1, C2) = capacities of its two experts (e1, e2):
    h13T_e = w13[e][half] @ xgT_e     # [I, Cx] rows of gate|up half
    hT_e   = silu(gate) * up         # [I/2, Cx]
    partT_e = w2[e][:, half] @ hT_e  # [D, Cx] partial (half contraction)
Everything is feature-major ([feature, token]) so both GEMMs use the weight
as the stationary operand and never need an on-device transpose.

Weights and activations are bf16 (fp32 PSUM accumulation). Measured
rooflines (per core): PE 384 matmuls x (C1+C2)/2 cols at the data-dependent
clock (~2.0-2.06 GHz under real bf16 toggle load due to the power/P0
governor; 2.4 GHz only on low-toggle data); HBM stream ~14.3 MB at
~352 GB/s. The kernel is PE-bound at the power-limited clock; fp8
(DoubleRow) would halve cycles but fails the 2e-2 accuracy gate (measured
8.5e-2 in simulation), so bf16 is the floor.

Input DMAs live on Sync/Scalar (HWDGE, alternating); under
For_i(staggered_reset) they prefetch iteration i+1 while iteration i's
GEMM2 is on the PE. UNROLL=4 with PE-only branch hints; wpool bufs=6.
"""

from contextlib import ExitStack

import ml_dtypes
import numpy as np

import concourse.bass as bass
import concourse.mybir as mybir
import concourse.tile as tile
from concourse import bacc
from concourse.bass_utils import run_bass_kernel_spmd

# Problem shape (hardcoded per harness contract).
E = 8          # experts
D = 1024       # model dim
I = 2048       # intermediate dim
P = 128        # SBUF partitions
KD = D // P    # 8 k-tiles over D
MPH = I // P // 2       # 8 gate|up pair-panels per I-half
NW13 = MPH // 2         # 4 w13 DMA tiles per expert-half (2 panels each)
MO = D // P             # 8 output row tiles
KIH = I // P // 2       # 8 k-tiles over the I-half
NW2 = MO // 2           # 4 w2 DMA tiles per expert-half (2 out-panels each)
PAIRC = KD * 2 * P      # columns of one gate|up fused panel
W13T = 2 * PAIRC        # columns of one w13 DMA tile (2 panels)
W2T = 2 * KIH * P       # columns of one w2 DMA tile (2 out-panels)

F32 = mybir.dt.float32
BF16 = mybir.dt.bfloat16
NP_BF16 = ml_dtypes.bfloat16


def build_program(C1: int, C2: int, repeats: int = 1, hw_loop: bool = False):
    """Build + compile the SPMD per-core program for capacities (C1, C2)."""
    nc = bacc.Bacc(
        "TRN2", target_bir_lowering=False, debug=False, num_devices=E
    )
    CS = C1 + C2
    xg_d = nc.dram_tensor("xg", [P, KD * CS], BF16, kind="ExternalInput").ap()
    w13p_d = nc.dram_tensor(
        "w13p", [P, 2 * NW13 * W13T], BF16, kind="ExternalInput"
    ).ap()
    w2p_d = nc.dram_tensor(
        "w2p", [P, 2 * NW2 * W2T], BF16, kind="ExternalInput"
    ).ap()
    out_d = nc.dram_tensor(
        "outt", [P, MO * CS], BF16, kind="ExternalOutput"
    ).ap()

    with tile.TileContext(nc) as tc, ExitStack() as ctx:
        resident = ctx.enter_context(tc.tile_pool(name="resident", bufs=1))
        psum = ctx.enter_context(tc.tile_pool(name="psum", bufs=4, space="PSUM"))
        spool = ctx.enter_context(tc.tile_pool(name="s", bufs=2))
        hpool = ctx.enter_context(tc.tile_pool(name="h", bufs=2))
        wpool = ctx.enter_context(tc.tile_pool(name="w", bufs=6))

        _emit_warmup(nc, resident, psum, spool, max(C1, C2))
        if hw_loop and repeats > 1:
            UNROLL = 4
            hint_engs = (mybir.EngineType.PE,)
            n_loop = (repeats - 1) // UNROLL
            leftover = repeats - n_loop * UNROLL
            if n_loop > 0:
                label = "moe_backedge"
                with tc.For_i(0, n_loop, 1,
                              hint_engines=hint_engs,
                              back_edge_label=label,
                              staggered_reset=True):
                    for u in range(UNROLL):
                        if u == UNROLL - 1:
                            tc.mark_branch_hint_location(
                                label, engines=hint_engs
                            )
                        _emit_body(nc, tc, hpool, wpool, psum, spool,
                                   xg_d, w13p_d, w2p_d, out_d, C1, C2)
            for _ in range(leftover):
                _emit_body(nc, tc, hpool, wpool, psum, spool,
                           xg_d, w13p_d, w2p_d, out_d, C1, C2)
        else:
            for _ in range(repeats):
                _emit_body(nc, tc, hpool, wpool, psum, spool,
                           xg_d, w13p_d, w2p_d, out_d, C1, C2)

    nc.compile()
    return nc


def _emit_warmup(nc, resident, psum, spool, C):
    # PE warm-up: dummy matmuls on a zeroed tile run while the first DMAs
    # land, pushing the PE clock-gate (HAM) to full rate before the real
    # matmul stream starts. A dummy Silu loads the ACT function table.
    warm = resident.tile([P, 512], BF16, tag="warm")
    nc.gpsimd.memset(warm[:], 0.0)
    pwarm = psum.tile([P, C], F32, tag="pg", bufs=2)
    for _ in range(8):
        nc.tensor.matmul(
            pwarm[:], lhsT=warm[:, :P], rhs=warm[:, :C], start=True, stop=True
        )
    sil_warm = spool.tile([P, 1], F32, tag="sil_warm")
    nc.scalar.activation(
        sil_warm[:], warm[:, :1], mybir.ActivationFunctionType.Silu
    )


def _emit_body(nc, tc, hpool, wpool, psum, spool,
               xg_d, w13p_d, w2p_d, out_d, C1, C2):
    CS = C1 + C2
    caps = (C1, C2)
    xg_off = (0, KD * C1)           # column offset of each expert's xg block
    hT_off = (0, KIH * C1)          # offset of each expert's hT block
    ot_off = (0, MO * C1)           # offset of each expert's output block

    hT_all = hpool.tile([P, KIH * CS], BF16, tag="hT_all")
    xgt = hpool.tile([P, KD * CS], BF16, tag="xgt")
    nc.sync.dma_start(out=xgt[:, : KD * C1], in_=xg_d[:, : KD * C1])
    nc.sync.dma_start(out=xgt[:, KD * C1 :], in_=xg_d[:, KD * C1 :])

    # GEMM1 + SiLU*up. 8 w13 DMA tiles: t in 0..3 -> expert 1 (free dim C1),
    # t in 4..7 -> expert 2 (free dim C2). Each tile = 2 fused gate|up panels.
    w2tiles = {}
    for t in range(8):
        ei = t // 4
        Cx = caps[ei]
        xb = xg_off[ei]
        wt = wpool.tile([P, W13T], BF16, tag="w13", bufs=8)
        if t == 0:
            # Split tile 0 across both HWDGE rings so its first half (and
            # the first matmuls) are ready sooner.
            nc.sync.dma_start(out=wt[:, :PAIRC], in_=w13p_d[:, :PAIRC])
            nc.scalar.dma_start(out=wt[:, PAIRC:], in_=w13p_d[:, PAIRC:W13T])
        else:
            eng = nc.sync if t % 2 == 0 else nc.scalar
            eng.dma_start(out=wt[:], in_=w13p_d[:, t * W13T : (t + 1) * W13T])
        for h in range(2):
            p = 2 * (t % 4) + h     # panel index within this expert's half
            base = h * PAIRC
            psg = psum.tile([P, Cx], F32, tag="pg", bufs=2)
            psu = psum.tile([P, Cx], F32, tag="pt", bufs=2)
            for k in range(KD):
                nc.tensor.matmul(
                    psg[:],
                    lhsT=wt[:, base + k * 2 * P : base + k * 2 * P + P],
                    rhs=xgt[:, xb + k * Cx : xb + (k + 1) * Cx],
                    start=(k == 0),
                    stop=(k == KD - 1),
                )
            for k in range(KD):
                nc.tensor.matmul(
                    psu[:],
                    lhsT=wt[:, base + k * 2 * P + P : base + (k + 1) * 2 * P],
                    rhs=xgt[:, xb + k * Cx : xb + (k + 1) * Cx],
                    start=(k == 0),
                    stop=(k == KD - 1),
                )
            sil = spool.tile([P, Cx], F32, tag="sil", bufs=3)
            nc.scalar.activation(
                sil[:], psg[:], mybir.ActivationFunctionType.Silu
            )
            nc.vector.tensor_mul(
                hT_all[:, hT_off[ei] + p * Cx : hT_off[ei] + (p + 1) * Cx],
                sil[:], psu[:],
            )

        # Prefetch the 8 w2 tiles (2 per t) during the back half of GEMM1.
        if t >= 4:
            for s in range(2):
                jo = 2 * (t - 4) + s
                w2t = wpool.tile([P, W2T], BF16, tag="w2", bufs=12)
                eng = nc.sync if jo % 2 == 0 else nc.scalar
                eng.dma_start(
                    out=w2t[:], in_=w2p_d[:, jo * W2T : (jo + 1) * W2T]
                )
                w2tiles[jo] = w2t

    # GEMM2 (half contraction): w2 tiles jo 0..3 -> expert 1, 4..7 -> expert 2.
    # Each tile = 2 out-panels x KIH k-slices. Partial outT goes to ot8.
    ot8 = spool.tile([P, MO * CS], BF16, tag="ot8", bufs=2)
    for ei in range(2):
        Cx = caps[ei]
        hb = hT_off[ei]
        ob = ot_off[ei]
        for mo in range(MO):
            jo = 4 * ei + mo // 2
            h = mo % 2
            w2t = w2tiles[jo]
            base = h * KIH * P
            ps2 = psum.tile([P, Cx], F32, tag="po", bufs=4)
            for ki in range(KIH):
                nc.tensor.matmul(
                    ps2[:],
                    lhsT=w2t[:, base + ki * P : base + (ki + 1) * P],
                    rhs=hT_all[:, hb + ki * Cx : hb + (ki + 1) * Cx],
                    start=(ki == 0),
                    stop=(ki == KIH - 1),
                )
            nc.vector.tensor_copy(
                ot8[:, ob + mo * Cx : ob + (mo + 1) * Cx], ps2[:]
            )
        # Ship each expert's partial as soon as its evacuations finish:
        # expert 1's DMA drains during expert 2's GEMM2, so the scalar ring
        # is nearly empty at the iteration boundary (where next iteration's
        # w13 tile-0 half is queued behind it).
        nc.scalar.dma_start(
            out=out_d[:, ob : ob + MO * Cx],
            in_=ot8[:, ob : ob + MO * Cx],
        )


def _pack_xg(x, tok, C):
    """Feature-major k-tiled activation image [P, KD*C] for a token set."""
    xg = np.zeros((D, C), dtype=NP_BF16)
    if len(tok):
        xg[:, : len(tok)] = x[tok].T.astype(NP_BF16)
    return np.ascontiguousarray(
        xg.reshape(KD, P, C).transpose(1, 0, 2).reshape(P, KD * C)
    )


def _w13_half_tiles(w13e, h):
    """4 DMA tiles [P, W13T] for half h of one expert's w13 (fused gate|up)."""
    w13t = w13e.T.astype(NP_BF16)                    # [D, 2I]
    a = w13t.reshape(KD, P, 2 * (I // P), P)
    mp = I // P                                       # 16 gate (and up) panels
    fused = np.concatenate([a[:, :, :mp, :], a[:, :, mp:, :]], axis=-1)
    fused = fused.transpose(2, 1, 0, 3).reshape(mp, P, KD * 2 * P)
    half = fused[h * MPH : (h + 1) * MPH]             # 8 pair-panels
    return np.ascontiguousarray(
        half.reshape(NW13, 2, P, KD * 2 * P)
        .transpose(0, 2, 1, 3)
        .reshape(NW13, P, W13T)
    )


def _w2_half_tiles(w2e, h):
    """4 DMA tiles [P, W2T] for half h of one expert's w2."""
    w2t = w2e.T.astype(NP_BF16)                      # [I, D]
    half = w2t[h * (I // 2) : (h + 1) * (I // 2)]    # [I/2, D]
    b = half.reshape(KIH, P, MO, P)
    pan = b.transpose(2, 1, 0, 3).reshape(MO, P, KIH * P)
    return np.ascontiguousarray(
        pan.reshape(NW2, 2, P, KIH * P)
        .transpose(0, 2, 1, 3)
        .reshape(NW2, P, W2T)
    )


def prepare_core_inputs(x, expert_indices, w13, w2):
    """Host-side routing + TP2 packing.

    Returns (in_maps, pairs, slot_lists, C1, C2). Pair j = (e1, e2) lives on
    cores (2j, 2j+1); core 2j holds I-half 0, core 2j+1 I-half 1. Expert e1
    uses free dim C1, e2 uses C2.
    """
    x = np.asarray(x)
    flat_e = np.asarray(expert_indices).reshape(-1).astype(np.int64)
    slot_lists = [np.nonzero(flat_e == e)[0] for e in range(E)]
    counts = np.array([len(s) for s in slot_lists])
    order = np.argsort(-counts, kind="stable")
    # Greedy big-with-small pairing: (1st,8th), (2nd,7th), ... The small
    # expert goes first (class C1), the big one second (class C2).
    pairs = [(int(order[E - 1 - j]), int(order[j])) for j in range(E // 2)]
    align = lambda n: max(2, ((n + 1) // 2) * 2)
    C1 = align(max(counts[e1] for e1, _ in pairs))
    C2 = align(max(counts[e2] for _, e2 in pairs))

    w13 = np.asarray(w13)
    w2 = np.asarray(w2)
    in_maps = []
    for j in range(E // 2):
        e1, e2 = pairs[j]
        tok1 = slot_lists[e1] // 2
        tok2 = slot_lists[e2] // 2
        xg = np.concatenate(
            [_pack_xg(x, tok1, C1), _pack_xg(x, tok2, C2)], axis=1
        )
        for h in range(2):
            w13p = np.concatenate(
                [_w13_half_tiles(w13[e1], h), _w13_half_tiles(w13[e2], h)],
                axis=0,
            )  # [8, P, W13T]
            w13p = np.ascontiguousarray(
                w13p.transpose(1, 0, 2).reshape(P, 8 * W13T)
            )
            w2p = np.concatenate(
                [_w2_half_tiles(w2[e1], h), _w2_half_tiles(w2[e2], h)],
                axis=0,
            )  # [8, P, W2T]
            w2p = np.ascontiguousarray(
                w2p.transpose(1, 0, 2).reshape(P, 8 * W2T)
            )
            in_maps.append({"xg": xg, "w13p": w13p, "w2p": w2p})
    return in_maps, pairs, slot_lists, C1, C2


def assemble_output(results, pairs, slot_lists, C1, C2, T, dtype):
    out = np.zeros((T, D), dtype=dtype)
    caps = (C1, C2)
    for j in range(len(pairs)):
        lo = np.asarray(results[2 * j]["outt"], dtype=np.float32)
        hi = np.asarray(results[2 * j + 1]["outt"], dtype=np.float32)
        full = lo + hi                                  # [P, MO*(C1+C2)]
        off = 0
        for ei, e in enumerate(pairs[j]):
            Cx = caps[ei]
            slots = slot_lists[e]
            blk = full[:, off : off + MO * Cx]
            outt = blk.reshape(P, MO, Cx).transpose(1, 0, 2).reshape(D, Cx)
            if len(slots):
                out[slots] = outt[:, : len(slots)].T.astype(dtype)
            off += MO * Cx
    return out


def kernel(x, expert_indices, w13, w2):
    x = np.asarray(x)
    idx = np.asarray(expert_indices)
    T = idx.size
    flat_e = idx.reshape(-1).astype(np.int64)
    max_n = max(1, max((flat_e == e).sum() for e in range(E)))
    if max_n > 512:
        # Pathological imbalance: PSUM limits one pass to 512 tokens/expert.
        # Run the fixed-capacity program once per <=512-sized chunk round.
        out = np.zeros((T, D), dtype=x.dtype)
        slot_lists = [np.nonzero(flat_e == e)[0] for e in range(E)]
        chunked = [
            [s[i : i + 512] for i in range(0, max(len(s), 1), 512)]
            for s in slot_lists
        ]
        rounds = max(len(c) for c in chunked)
        for r in range(rounds):
            flat = np.full(T, -1, dtype=np.int64)
            for e, c in enumerate(chunked):
                if r < len(c):
                    flat[c[r]] = e
            sub_idx = flat.reshape(idx.shape)
            in_maps, pairs, sub_lists, C1, C2 = prepare_core_inputs(
                x, sub_idx, w13, w2
            )
            nc = build_program(C1, C2)
            res = _run_with_retry(nc, in_maps)
            part = assemble_output(
                res.results, pairs, sub_lists, C1, C2, T, x.dtype
            )
            mask = flat >= 0
            out[mask] = part[mask]
        return out
    in_maps, pairs, slot_lists, C1, C2 = prepare_core_inputs(
        x, idx, w13, w2
    )
    nc = build_program(C1, C2)
    res = _run_with_retry(nc, in_maps)
    return assemble_output(
        res.results, pairs, slot_lists, C1, C2, T, x.dtype
    )


def _run_with_retry(nc, in_maps, attempts=3):
    last_err = None
    for _ in range(attempts):
        try:
            return run_bass_kernel_spmd(nc, in_maps, core_ids=list(range(E)))
        except Exception as exc:  # intermittent NRT exec-unit wedge: retry
            last_err = exc
    raise last_err
